# revision 5
# baseline (speedup 1.0000x reference)
"""Trainium2 Bass kernel for nn_EntInit (gnn_message_passing).

feat[n, :] = mean over incoming edges e (dst[e] == n) of T[etypes[e], :]
where T = concat(rel_head_emb, rel_tail_emb)[etype].

Strategy: the whole segment reduction runs on the PE via one-hot
matmuls — no DMA gather/scatter (descriptor-rate-bound on gpsimd
software DGE in the earlier version of this kernel).

  - Nodes are split into 64-node blocks; each core owns 98 contiguous
    blocks. Edges are routed (host side, index math only) to their
    block, split into NQ=4 interleaved type groups (q = etype % 4,
    row r = etype // 4 < 100), padded to 128-edge tiles: K tiles per
    (block, q) group, K = global max (SPMD-static).
  - Per tile (128 edges): two tensor_scalar(is_equal) ops — split
    across DVE and gpsimd to balance engine load — build one-hot
    matrices A[e, rr] = (r_e == rr) (100 wide) and B[e, n] =
    (dst%64 == n) (64 wide) against a constant iota row tile; one PE
    matmul accumulates CT[r, n] += A^T B into PSUM over the block's K
    tiles (CT = per-block [type-row, node] edge-count histogram,
    exact small ints).
  - Per block: CT (bf16, exact) x relation-table matmuls accumulate
    sums[n, 0:128] and counts (table carries an all-ones column), using
    bf16 hi+lo table splits for ~f32 precision; ACT scales by
    1/max(count,1); DMA out.

Padding edges carry sentinel -1 which matches no one-hot column and
thus contributes nothing anywhere.
"""
import sys

sys.path.insert(0, "/opt/trn_rl_repo")

import numpy as np
import ml_dtypes

import concourse.bass as bass
import concourse.bacc as bacc
import concourse.mybir as mybir
import concourse.tile as tile

NUM_REL = 200
N_TYPES = 2 * NUM_REL          # 400 relation rows
N_CORES = 8
P = 128
NQ = 4                         # interleaved type chunks (400 types -> 4x100)
RW = 100                       # A one-hot width: r = etype // 4 in [0, 100)
BP = 64                        # nodes per block (B one-hot width)
NBC = 98                       # node blocks per core (8*98*64 = 50176 >= 50000)
N_NODES = 50000
CW = 258                       # table cols per q chunk: 129 hi|ones + 128 lo + 1 zero
BF16 = ml_dtypes.bfloat16

_prog_cache: dict = {}
_runner_cache: dict = {}


def _build_program(K: int):
    """One SPMD program; cores differ only in input data.

    K = tiles per (block, q) group. Per core: NBC blocks x NQ q-groups x K
    128-edge tiles.
    """
    TBLK = NQ * K                  # tiles per block
    TANT = NBC * TBLK              # tiles per core
    nc = bacc.Bacc("TRN2", debug=False, num_devices=1)
    colsd = nc.dram_tensor("cols", [P, TANT * 2], mybir.dt.int8,
                           kind="ExternalInput").ap()
    wtd = nc.dram_tensor("wt", [P, NQ * CW], mybir.dt.bfloat16,
                         kind="ExternalInput").ap()
    iod = nc.dram_tensor("iota", [P, P], mybir.dt.bfloat16,
                         kind="ExternalInput").ap()
    featd = nc.dram_tensor("feat", [NBC * BP, P], mybir.dt.float16,
                           kind="ExternalOutput").ap()

    with tile.TileContext(nc) as tc:
        with (
            tc.tile_pool(name="const", bufs=1) as const_tp,
            tc.tile_pool(name="cin", bufs=3) as cin_tp,
            tc.tile_pool(name="oh", bufs=6) as oh_tp,
            tc.tile_pool(name="ctsb", bufs=2) as ctsb_tp,
            tc.tile_pool(name="eps", bufs=2) as eps_tp,
            tc.tile_pool(name="psct", bufs=2, space="PSUM") as psct_tp,
            tc.tile_pool(name="pssum", bufs=2, space="PSUM") as pssum_tp,
        ):
            wt_sb = const_tp.tile([P, NQ * CW], mybir.dt.bfloat16)
            nc.sync.dma_start(out=wt_sb[:], in_=wtd[:])
            io_sb = const_tp.tile([P, P], mybir.dt.bfloat16)
            nc.sync.dma_start(out=io_sb[:], in_=iod[:])

            def emit_tail(b, ct_ps):
                """Finish block b: table matmuls + normalize + store."""
                ct_sb = ctsb_tp.tile([RW, NQ * BP], mybir.dt.bfloat16, tag="ctsb")
                nc.scalar.copy(out=ct_sb[:], in_=ct_ps[:])
                sums = pssum_tp.tile([BP, 129], mybir.dt.float32, tag="sums")
                for q in range(NQ):
                    nc.tensor.matmul(
                        out=sums[:], lhsT=ct_sb[:, q * BP:(q + 1) * BP],
                        rhs=wt_sb[0:RW, q * CW:q * CW + 129],
                        start=(q == 0), stop=False,
                    )
                    nc.tensor.matmul(
                        out=sums[:], lhsT=ct_sb[:, q * BP:(q + 1) * BP],
                        rhs=wt_sb[0:RW, q * CW + 129:(q + 1) * CW],
                        start=False, stop=(q == NQ - 1),
                    )
                cm = eps_tp.tile([BP, 1], mybir.dt.float32, tag="cm")
                nc.vector.tensor_scalar_max(out=cm[:], in0=sums[:, 128:129],
                                            scalar1=1.0)
                rc = eps_tp.tile([BP, 1], mybir.dt.float32, tag="rc")
                nc.vector.reciprocal(out=rc[:], in_=cm[:])
                ft = eps_tp.tile([BP, P], mybir.dt.float16, tag="ft")
                nc.scalar.mul(out=ft[:], in_=sums[:, 0:128], mul=rc[:])
                nc.sync.dma_start(out=featd[b * BP:(b + 1) * BP, :], in_=ft[:])

            prev = None
            for b in range(NBC):
                cin8 = cin_tp.tile([P, TBLK * 2], mybir.dt.int8, tag="cin8")
                nc.sync.dma_start(
                    out=cin8[:], in_=colsd[:, b * TBLK * 2:(b + 1) * TBLK * 2])
                cin = cin_tp.tile([P, TBLK * 2], mybir.dt.float32, tag="cin")
                nc.scalar.copy(out=cin[:], in_=cin8[:])
                ct_ps = psct_tp.tile([RW, NQ * BP], mybir.dt.float32, tag="ct")
                for q in range(NQ):
                    for j in range(K):
                        t = q * K + j
                        oh = oh_tp.tile([P, 2, P], mybir.dt.bfloat16, tag="oh")
                        # balance the one-hot stream across DVE and gpsimd
                        aeng = nc.gpsimd if t % 5 == 4 else nc.vector
                        beng = nc.gpsimd
                        aeng.tensor_scalar(
                            out=oh[:, 0, 0:RW], in0=io_sb[:, 0:RW],
                            scalar1=cin[:, 2 * t:2 * t + 1], scalar2=None,
                            op0=mybir.AluOpType.is_equal)
                        beng.tensor_scalar(
                            out=oh[:, 1, 0:BP], in0=io_sb[:, 0:BP],
                            scalar1=cin[:, 2 * t + 1:2 * t + 2], scalar2=None,
                            op0=mybir.AluOpType.is_equal)
                        nc.tensor.matmul(
                            out=ct_ps[:, q * BP:(q + 1) * BP],
                            lhsT=oh[:, 0, 0:RW], rhs=oh[:, 1, 0:BP],
                            start=(j == 0), stop=(j == K - 1),
                        )
                if prev is not None:
                    emit_tail(*prev)
                prev = (b, ct_ps)
            emit_tail(*prev)

    nc.compile()
    return nc


def _host_prepare(et: np.ndarray, d: np.ndarray):
    """Route edges to (core, block, q, tile, slot); sentinel-pad. Index
    math only — all numerics happen on device."""
    E = et.shape[0]
    # interleaved type split: type tau -> (q = tau % NQ, r = tau // NQ) so
    # the NQ groups get equal type counts (100 each) -> balanced tiles
    q_e = (et & (NQ - 1)).astype(np.int64)
    r_e = (et >> 2).astype(np.int64)
    dl_e = (d & (BP - 1)).astype(np.int64)
    blk = (d >> 6).astype(np.int64)

    G = NBC * N_CORES * NQ
    grp = blk * NQ + q_e
    cnt = np.bincount(grp, minlength=G)
    K = int(-(-cnt.max() // P))
    # in-degree cap so CT counts stay exact in bf16
    assert np.bincount(d, minlength=N_NODES).max() <= 255

    order = np.argsort(grp, kind="stable")
    starts = np.zeros(G + 1, np.int64)
    np.cumsum(cnt, out=starts[1:])
    g_s = grp[order]
    pos = np.arange(E, dtype=np.int64) - starts[g_s]

    blk_s = g_s // NQ
    q_s = g_s - blk_s * NQ
    core_s = blk_s // NBC
    blkl_s = blk_s - core_s * NBC
    tile_local = (blkl_s * NQ + q_s) * K + (pos >> 7)
    slot = pos & 127

    TANT = NBC * NQ * K
    cols = np.full((N_CORES, P, TANT, 2), -1, np.int8)
    cols[core_s, slot, tile_local, 0] = r_e[order]
    cols[core_s, slot, tile_local, 1] = dl_e[order]
    cols_f = np.ascontiguousarray(cols.reshape(N_CORES, P, TANT * 2))
    return cols_f, K


def _make_table(head: np.ndarray, tail: np.ndarray) -> np.ndarray:
    W = np.concatenate([head, tail], axis=0).astype(np.float32)  # [400, 128]
    # chunk q holds types tau with tau % NQ == q at row r = tau // NQ
    wt = np.zeros((P, NQ * CW), BF16)
    for q in range(NQ):
        taus = np.arange(q, N_TYPES, NQ)          # types in this chunk
        rows = taus // NQ                          # their r rows
        sub = W[taus]                              # [len, 128] f32
        hi = sub.astype(BF16)
        lo = (sub - hi.astype(np.float32)).astype(BF16)
        wt[rows, q * CW:q * CW + 128] = hi
        wt[rows, q * CW + 128] = BF16(1.0)
        wt[rows, q * CW + 129:q * CW + 257] = lo
    return wt


def _make_iota() -> np.ndarray:
    return np.broadcast_to(np.arange(P, dtype=np.float32), (P, P)).astype(BF16)


def _get_runner(nc):
    """Cached jitted SPMD executor (mirrors bass2jax.run_bass_via_pjrt's
    multi-core branch, but reusable across calls without re-tracing)."""
    key = id(nc)
    if key in _runner_cache:
        return _runner_cache[key]
    import jax
    from jax.experimental.shard_map import shard_map
    from jax.sharding import Mesh, PartitionSpec
    from concourse import bass2jax
    from concourse.bass2jax import _bass_exec_p, partition_id_tensor

    bass2jax.install_neuronx_cc_hook()

    in_names, out_names, out_avals, zero_shapes = [], [], [], []
    for alloc in nc.m.functions[0].allocations:
        if not isinstance(alloc, mybir.MemoryLocationSet):
            continue
        name = alloc.memorylocations[0].name
        if alloc.kind == "ExternalInput":
            if nc.partition_id_tensor is None or name != nc.partition_id_tensor.name:
                in_names.append(name)
        elif alloc.kind == "ExternalOutput":
            shape = tuple(alloc.tensor_shape)
            dtype = mybir.dt.np(alloc.dtype)
            out_names.append(name)
            out_avals.append(jax.core.ShapedArray(shape, dtype))
            zero_shapes.append((shape, dtype))
    n_params = len(in_names)
    all_names = list(in_names) + list(out_names)
    if nc.partition_id_tensor is not None:
        all_names.append(nc.partition_id_tensor.name)
    donate = tuple(range(n_params, n_params + len(out_names)))

    def _body(*args):
        operands = list(args)
        if nc.partition_id_tensor is not None:
            operands.append(partition_id_tensor())
        outs = _bass_exec_p.bind(
            *operands,
            out_avals=tuple(out_avals),
            in_names=tuple(all_names),
            out_names=tuple(out_names),
            lowering_input_output_aliases=(),
            sim_require_finite=True,
            sim_require_nnan=True,
            nc=nc,
        )
        return tuple(outs)

    devices = jax.devices()[:N_CORES]
    mesh = Mesh(np.asarray(devices), ("core",))
    in_specs = (PartitionSpec("core"),) * (n_params + len(out_names))
    out_specs = (PartitionSpec("core"),) * len(out_names)
    fn = jax.jit(
        shard_map(_body, mesh=mesh, in_specs=in_specs, out_specs=out_specs,
                  check_rep=False),
        donate_argnums=donate, keep_unused=True,
    )
    r = (fn, in_names, out_names, out_avals, zero_shapes)
    _runner_cache[key] = r
    return r


class _Res:
    def __init__(self, results):
        self.results = results


def _run_spmd_cached(nc, in_maps):
    fn, in_names, out_names, out_avals, zero_shapes = _get_runner(nc)
    concat_in = [np.concatenate([m[n] for m in in_maps], axis=0) for n in in_names]
    concat_zeros = [np.zeros((N_CORES * s[0], *s[1:]), d) for s, d in zero_shapes]
    out_arrs = fn(*concat_in, *concat_zeros)
    results = []
    for c in range(N_CORES):
        results.append({
            name: np.asarray(out_arrs[i]).reshape(N_CORES, *out_avals[i].shape)[c]
            for i, name in enumerate(out_names)
        })
    return _Res(results)


def kernel(etypes, dst, rel_head_emb, rel_tail_emb, n_nodes):
    et = np.asarray(etypes).astype(np.int64)
    d = np.asarray(dst).astype(np.int64)
    head = np.asarray(rel_head_emb, dtype=np.float32)
    tail = np.asarray(rel_tail_emb, dtype=np.float32)
    nn = int(n_nodes)
    assert nn == N_NODES, f"compiled for {N_NODES} nodes, got {nn}"

    cols_bf, K = _host_prepare(et, d)
    wt = _make_table(head, tail)
    io = _make_iota()
    in_maps = [{"cols": cols_bf[k], "wt": wt, "iota": io}
               for k in range(N_CORES)]

    if K not in _prog_cache:
        _prog_cache[K] = _build_program(K)
    nc = _prog_cache[K]

    res = _run_spmd_cached(nc, in_maps)

    out = np.zeros((nn, P), np.float32)
    npc = NBC * BP
    for k in range(N_CORES):
        lo_n = k * npc
        hi_n = min((k + 1) * npc, nn)
        if lo_n >= nn:
            break
        out[lo_n:hi_n] = res.results[k]["feat"][0:hi_n - lo_n].astype(np.float32)
    return out


# revision 6
# speedup vs baseline: 2.6525x; 2.6525x over previous
"""Trainium2 Bass kernel for nn_EntInit (gnn_message_passing).

feat[n, :] = mean over incoming edges e (dst[e] == n) of T[etypes[e], :]
where T = concat(rel_head_emb, rel_tail_emb)[etype].

Strategy: the whole segment reduction runs on the PE via one-hot
matmuls — no DMA gather/scatter (descriptor-rate-bound on gpsimd
software DGE in the first version of this kernel).

  - Nodes are split into 64-node blocks; each core owns 98 contiguous
    blocks. Edges are routed (host side, index math only) to their
    block, split into NQ=5 interleaved type groups (q = etype % 5,
    row r = etype // 5 < 80), padded to 128-edge tiles: K tiles per
    (block, q) group, K = global max (SPMD-static).
  - Per tile (128 edges): two tensor_scalar(is_equal) ops — split
    across DVE and gpsimd to balance engine load — build one-hot
    matrices A[e, rr] = (r_e == rr) (80 wide) and B[e, n] =
    (dst%64 == n) (64 wide) against a constant iota row tile; one PE
    matmul accumulates
    CT[r, n] += A^T B into PSUM over the block's K tiles (CT = per-block
    [type-remainder, node] edge-count histogram, exact small ints).
  - Per block: CT (bf16, exact) x relation-table matmuls accumulate
    sums[n, 0:128] and counts (table carries an all-ones column), using
    bf16 hi+lo table splits for ~f32 precision; ACT scales by
    1/max(count,1); DMA out.

Padding edges carry sentinel -1 which matches no one-hot column and
thus contributes nothing anywhere.
"""
import sys

sys.path.insert(0, "/opt/trn_rl_repo")

import numpy as np
import ml_dtypes

import concourse.bass as bass
import concourse.bacc as bacc
import concourse.mybir as mybir
import concourse.tile as tile

NUM_REL = 200
N_TYPES = 2 * NUM_REL          # 400 relation rows
N_CORES = 8
P = 128
NQ = 5                         # interleaved type chunks (400 types -> 5x80)
RW = 80                        # A one-hot width: r = etype // 5 in [0, 80)
BP = 64                        # nodes per block (B one-hot width)
NBC = 98                       # node blocks per core (8*98*64 = 50176 >= 50000)
N_NODES = 50000
CW = 258                       # table cols per q chunk: 129 hi|ones + 128 lo + 1 zero
BF16 = ml_dtypes.bfloat16

_prog_cache: dict = {}
_runner_cache: dict = {}


def _build_program(K: int):
    """One SPMD program; cores differ only in input data.

    K = tiles per (block, q) group. Per core: NBC blocks x NQ q-groups x K
    128-edge tiles.
    """
    TBLK = NQ * K                  # tiles per block
    TANT = NBC * TBLK              # tiles per core
    nc = bacc.Bacc("TRN2", debug=False, num_devices=1)
    colsd = nc.dram_tensor("cols", [P, TANT * 2], mybir.dt.int8,
                           kind="ExternalInput").ap()
    wtd = nc.dram_tensor("wt", [P, NQ * CW], mybir.dt.bfloat16,
                         kind="ExternalInput").ap()
    iod = nc.dram_tensor("iota", [P, P], mybir.dt.bfloat16,
                         kind="ExternalInput").ap()
    featd = nc.dram_tensor("feat", [NBC * BP, P], mybir.dt.float16,
                           kind="ExternalOutput").ap()

    with tile.TileContext(nc) as tc:
        with (
            tc.tile_pool(name="const", bufs=1) as const_tp,
            tc.tile_pool(name="cin", bufs=3) as cin_tp,
            tc.tile_pool(name="oh", bufs=6) as oh_tp,
            tc.tile_pool(name="ctsb", bufs=2) as ctsb_tp,
            tc.tile_pool(name="eps", bufs=2) as eps_tp,
            tc.tile_pool(name="psct", bufs=2, space="PSUM") as psct_tp,
            tc.tile_pool(name="pssum", bufs=2, space="PSUM") as pssum_tp,
        ):
            wt_sb = const_tp.tile([P, NQ * CW], mybir.dt.bfloat16)
            nc.sync.dma_start(out=wt_sb[:], in_=wtd[:])
            io_sb = const_tp.tile([P, P], mybir.dt.bfloat16)
            nc.sync.dma_start(out=io_sb[:], in_=iod[:])

            def emit_tail(b, ct_ps):
                """Finish block b: table matmuls + normalize + store."""
                ct_sb = ctsb_tp.tile([RW, NQ * BP], mybir.dt.bfloat16, tag="ctsb")
                nc.scalar.copy(out=ct_sb[:], in_=ct_ps[:])
                sums = pssum_tp.tile([BP, 129], mybir.dt.float32, tag="sums")
                for q in range(NQ):
                    nc.tensor.matmul(
                        out=sums[:], lhsT=ct_sb[:, q * BP:(q + 1) * BP],
                        rhs=wt_sb[0:RW, q * CW:q * CW + 129],
                        start=(q == 0), stop=False,
                    )
                    nc.tensor.matmul(
                        out=sums[:], lhsT=ct_sb[:, q * BP:(q + 1) * BP],
                        rhs=wt_sb[0:RW, q * CW + 129:(q + 1) * CW],
                        start=False, stop=(q == NQ - 1),
                    )
                cm = eps_tp.tile([BP, 1], mybir.dt.float32, tag="cm")
                nc.vector.tensor_scalar_max(out=cm[:], in0=sums[:, 128:129],
                                            scalar1=1.0)
                rc = eps_tp.tile([BP, 1], mybir.dt.float32, tag="rc")
                nc.vector.reciprocal(out=rc[:], in_=cm[:])
                ft = eps_tp.tile([BP, P], mybir.dt.float16, tag="ft")
                nc.scalar.mul(out=ft[:], in_=sums[:, 0:128], mul=rc[:])
                nc.sync.dma_start(out=featd[b * BP:(b + 1) * BP, :], in_=ft[:])

            prev = None
            for b in range(NBC):
                cin8 = cin_tp.tile([P, TBLK * 2], mybir.dt.int8, tag="cin8")
                nc.sync.dma_start(
                    out=cin8[:], in_=colsd[:, b * TBLK * 2:(b + 1) * TBLK * 2])
                cin = cin_tp.tile([P, TBLK * 2], mybir.dt.float32, tag="cin")
                nc.scalar.copy(out=cin[:], in_=cin8[:])
                ct_ps = psct_tp.tile([RW, NQ * BP], mybir.dt.float32, tag="ct")
                for q in range(NQ):
                    for j in range(K):
                        t = q * K + j
                        oh = oh_tp.tile([P, 2, P], mybir.dt.bfloat16, tag="oh")
                        # balance the one-hot stream across DVE and gpsimd
                        aeng = nc.gpsimd if t % 5 == 4 else nc.vector
                        beng = nc.gpsimd
                        aeng.tensor_scalar(
                            out=oh[:, 0, 0:RW], in0=io_sb[:, 0:RW],
                            scalar1=cin[:, 2 * t:2 * t + 1], scalar2=None,
                            op0=mybir.AluOpType.is_equal)
                        beng.tensor_scalar(
                            out=oh[:, 1, 0:BP], in0=io_sb[:, 0:BP],
                            scalar1=cin[:, 2 * t + 1:2 * t + 2], scalar2=None,
                            op0=mybir.AluOpType.is_equal)
                        nc.tensor.matmul(
                            out=ct_ps[:, q * BP:(q + 1) * BP],
                            lhsT=oh[:, 0, 0:RW], rhs=oh[:, 1, 0:BP],
                            start=(j == 0), stop=(j == K - 1),
                        )
                if prev is not None:
                    emit_tail(*prev)
                prev = (b, ct_ps)
            emit_tail(*prev)

    nc.compile()
    return nc


def _host_prepare(et: np.ndarray, d: np.ndarray):
    """Route edges to (core, block, q, tile, slot); sentinel-pad. Index
    math only — all numerics happen on device."""
    E = et.shape[0]
    # interleaved type split: type tau -> (q = tau % NQ, r = tau // NQ) so
    # the NQ groups get equal type counts (100 each) -> balanced tiles
    q_e = (et % NQ).astype(np.int64)
    r_e = (et // NQ).astype(np.int64)
    dl_e = (d & (BP - 1)).astype(np.int64)
    blk = (d >> 6).astype(np.int64)

    G = NBC * N_CORES * NQ
    grp = blk * NQ + q_e
    cnt = np.bincount(grp, minlength=G)
    K = int(-(-cnt.max() // P))
    # in-degree cap so CT counts stay exact in bf16
    assert np.bincount(d, minlength=N_NODES).max() <= 255

    order = np.argsort(grp, kind="stable")
    starts = np.zeros(G + 1, np.int64)
    np.cumsum(cnt, out=starts[1:])
    g_s = grp[order]
    pos = np.arange(E, dtype=np.int64) - starts[g_s]

    blk_s = g_s // NQ
    q_s = g_s - blk_s * NQ
    core_s = blk_s // NBC
    blkl_s = blk_s - core_s * NBC
    tile_local = (blkl_s * NQ + q_s) * K + (pos >> 7)
    slot = pos & 127

    TANT = NBC * NQ * K
    cols = np.full((N_CORES, P, TANT, 2), -1, np.int8)
    cols[core_s, slot, tile_local, 0] = r_e[order]
    cols[core_s, slot, tile_local, 1] = dl_e[order]
    cols_f = np.ascontiguousarray(cols.reshape(N_CORES, P, TANT * 2))
    return cols_f, K


def _make_table(head: np.ndarray, tail: np.ndarray) -> np.ndarray:
    W = np.concatenate([head, tail], axis=0).astype(np.float32)  # [400, 128]
    # chunk q holds types tau with tau % NQ == q at row r = tau // NQ
    wt = np.zeros((P, NQ * CW), BF16)
    for q in range(NQ):
        taus = np.arange(q, N_TYPES, NQ)          # types in this chunk
        rows = taus // NQ                          # their r rows
        sub = W[taus]                              # [len, 128] f32
        hi = sub.astype(BF16)
        lo = (sub - hi.astype(np.float32)).astype(BF16)
        wt[rows, q * CW:q * CW + 128] = hi
        wt[rows, q * CW + 128] = BF16(1.0)
        wt[rows, q * CW + 129:q * CW + 257] = lo
    return wt


def _make_iota() -> np.ndarray:
    return np.broadcast_to(np.arange(P, dtype=np.float32), (P, P)).astype(BF16)


def _get_runner(nc):
    """Cached jitted SPMD executor (mirrors bass2jax.run_bass_via_pjrt's
    multi-core branch, but reusable across calls without re-tracing)."""
    key = id(nc)
    if key in _runner_cache:
        return _runner_cache[key]
    import jax
    from jax.experimental.shard_map import shard_map
    from jax.sharding import Mesh, PartitionSpec
    from concourse import bass2jax
    from concourse.bass2jax import _bass_exec_p, partition_id_tensor

    bass2jax.install_neuronx_cc_hook()

    in_names, out_names, out_avals, zero_shapes = [], [], [], []
    for alloc in nc.m.functions[0].allocations:
        if not isinstance(alloc, mybir.MemoryLocationSet):
            continue
        name = alloc.memorylocations[0].name
        if alloc.kind == "ExternalInput":
            if nc.partition_id_tensor is None or name != nc.partition_id_tensor.name:
                in_names.append(name)
        elif alloc.kind == "ExternalOutput":
            shape = tuple(alloc.tensor_shape)
            dtype = mybir.dt.np(alloc.dtype)
            out_names.append(name)
            out_avals.append(jax.core.ShapedArray(shape, dtype))
            zero_shapes.append((shape, dtype))
    n_params = len(in_names)
    all_names = list(in_names) + list(out_names)
    if nc.partition_id_tensor is not None:
        all_names.append(nc.partition_id_tensor.name)
    donate = tuple(range(n_params, n_params + len(out_names)))

    def _body(*args):
        operands = list(args)
        if nc.partition_id_tensor is not None:
            operands.append(partition_id_tensor())
        outs = _bass_exec_p.bind(
            *operands,
            out_avals=tuple(out_avals),
            in_names=tuple(all_names),
            out_names=tuple(out_names),
            lowering_input_output_aliases=(),
            sim_require_finite=True,
            sim_require_nnan=True,
            nc=nc,
        )
        return tuple(outs)

    devices = jax.devices()[:N_CORES]
    mesh = Mesh(np.asarray(devices), ("core",))
    in_specs = (PartitionSpec("core"),) * (n_params + len(out_names))
    out_specs = (PartitionSpec("core"),) * len(out_names)
    fn = jax.jit(
        shard_map(_body, mesh=mesh, in_specs=in_specs, out_specs=out_specs,
                  check_rep=False),
        donate_argnums=donate, keep_unused=True,
    )
    r = (fn, in_names, out_names, out_avals, zero_shapes)
    _runner_cache[key] = r
    return r


class _Res:
    def __init__(self, results):
        self.results = results


def _run_spmd_cached(nc, in_maps):
    fn, in_names, out_names, out_avals, zero_shapes = _get_runner(nc)
    concat_in = [np.concatenate([m[n] for m in in_maps], axis=0) for n in in_names]
    concat_zeros = [np.zeros((N_CORES * s[0], *s[1:]), d) for s, d in zero_shapes]
    out_arrs = fn(*concat_in, *concat_zeros)
    results = []
    for c in range(N_CORES):
        results.append({
            name: np.asarray(out_arrs[i]).reshape(N_CORES, *out_avals[i].shape)[c]
            for i, name in enumerate(out_names)
        })
    return _Res(results)


def kernel(etypes, dst, rel_head_emb, rel_tail_emb, n_nodes):
    et = np.asarray(etypes).astype(np.int64)
    d = np.asarray(dst).astype(np.int64)
    head = np.asarray(rel_head_emb, dtype=np.float32)
    tail = np.asarray(rel_tail_emb, dtype=np.float32)
    nn = int(n_nodes)
    assert nn == N_NODES, f"compiled for {N_NODES} nodes, got {nn}"

    cols_bf, K = _host_prepare(et, d)
    wt = _make_table(head, tail)
    io = _make_iota()
    in_maps = [{"cols": cols_bf[k], "wt": wt, "iota": io}
               for k in range(N_CORES)]

    if K not in _prog_cache:
        _prog_cache[K] = _build_program(K)
    nc = _prog_cache[K]

    res = _run_spmd_cached(nc, in_maps)

    out = np.zeros((nn, P), np.float32)
    npc = NBC * BP
    for k in range(N_CORES):
        lo_n = k * npc
        hi_n = min((k + 1) * npc, nn)
        if lo_n >= nn:
            break
        out[lo_n:hi_n] = res.results[k]["feat"][0:hi_n - lo_n].astype(np.float32)
    return out
